# revision 9
# baseline (speedup 1.0000x reference)
"""BVGAE GNN message-passing kernel for 8 TRN2 NeuronCores.

Pipeline (4 SPMD launches, host gathers the tiny cross-core activations
between launches; each launch is row-sharded across the 8 cores):

  L1  x1    : X1 = norm * (h @ W0.T)            (rows sharded, 1024/core)
  L2  spmm1 : S1 = A @ X1 via dma_gather + one-hot matmul scatter-add;
              h0s = norm * relu(norm * S1); also projects the two head
              scalars T = h0s @ (W1.T @ w_{alpha,beta}) per node
  L3  spmm2 : x_ab = norm * (A @ T) + b; elu(x)+1.5  (slim 256B gathers)
  L4  pairs : alpha_p/beta_p row-panels: out[i,j] = a[i] + a[j] (bf16,
              host upconverts to f32)

Math identity used for layer 2: h1 @ w = norm * (A @ (h0s @ W1.T)) @ w
= norm * (A @ (h0s @ (W1.T @ w))), so the second SPMM only needs the
2-wide projected table T instead of the 256-wide h0s table.

The `reps` builder argument repeats the launch body back-to-back inside
one NEFF; the test harness uses it to slope-fit per-launch HW time
(wall-clock can't resolve microseconds through the PJRT proxy).
"""
import os
import numpy as np

import concourse.bass as bass
import concourse.bacc as bacc
import concourse.mybir as mybir
import concourse.tile as tile
from concourse.bass_utils import run_bass_kernel_spmd

F32 = mybir.dt.float32
BF16 = mybir.dt.bfloat16
I16 = mybir.dt.int16
NP_BF16 = mybir.dt.np(BF16)
AOT = mybir.AluOpType
AFT = mybir.ActivationFunctionType
MS = bass.MemorySpace

N = 8192
F_IN = 512
HID = 256
CORES = 8
RPC = N // CORES          # rows per core
TPC = RPC // 128          # 128-row tiles per core
OBATCH = 2                # L4: 128-row tiles batched per output DMA
TSLIM = 128               # L3: bf16 table width (256B gather elements)

LAST_LAUNCHES = []        # (name, builder, in_maps) stashed when BVGAE_KEEP=1


def _run(nc, in_maps, name="", builder=None):
    if os.environ.get("BVGAE_KEEP") == "1":
        LAST_LAUNCHES.append((name, builder, in_maps))
    res = run_bass_kernel_spmd(nc, in_maps, core_ids=list(range(CORES)))
    return res.results


def _norm_tiles(nc, pool, deg_dram):
    """deg [128, TPC] -> norm = 1/sqrt(deg) in SBUF [128, TPC]."""
    deg_sb = pool.tile([128, TPC], F32)
    nc.sync.dma_start(deg_sb[:], deg_dram[:])
    sq = pool.tile([128, TPC], F32)
    nc.scalar.activation(sq[:], deg_sb[:], AFT.Sqrt)
    norm_sb = pool.tile([128, TPC], F32)
    nc.vector.reciprocal(norm_sb[:], sq[:])
    return norm_sb


def _build_l1(reps=1):
    nc = bacc.Bacc("TRN2", target_bir_lowering=False, debug=False,
                   num_devices=CORES)
    KB = F_IN // 128
    ht = nc.dram_tensor("ht", [KB, 128, RPC], BF16, kind="ExternalInput")
    w0t = nc.dram_tensor("w0t", [KB, 128, HID], BF16, kind="ExternalInput")
    deg = nc.dram_tensor("degc", [128, TPC], F32, kind="ExternalInput")
    x1c = nc.dram_tensor("x1c", [RPC, HID], BF16, kind="ExternalOutput")

    with tile.TileContext(nc) as tc:
        with (
            tc.tile_pool(name="pool", bufs=2) as pool,
            tc.tile_pool(name="io", bufs=2) as io,
            tc.tile_pool(name="psum", bufs=2, space=MS.PSUM) as pps,
        ):
            for _rep in range(reps):
                ht_sb = pool.tile([128, KB, RPC], BF16, tag="ht")
                w0_sb = pool.tile([128, KB, HID], BF16, tag="w0")
                for k in range(KB):
                    nc.sync.dma_start(ht_sb[:, k, :], ht[k])
                    nc.sync.dma_start(w0_sb[:, k, :], w0t[k])
                norm_sb = _norm_tiles(nc, pool, deg)

                for t in range(TPC):
                    ps = pps.tile([128, HID], F32, tag="ps")
                    for k in range(KB):
                        nc.tensor.matmul(
                            ps[:], ht_sb[:, k, bass.ts(t, 128)],
                            w0_sb[:, k, :],
                            start=(k == 0), stop=(k == KB - 1),
                        )
                    xt = io.tile([128, HID], BF16, tag="xt")
                    nc.scalar.activation(xt[:], ps[:], AFT.Copy,
                                         scale=norm_sb[:, t:t + 1])
                    nc.sync.dma_start(x1c[bass.ts(t, 128), :], xt[:])
    nc.compile()
    return nc


def _build_l2(nblk, reps=1):
    """SPMM layer 1 + head projection: from the bf16 X1 table, compute
    h0s rows for this core plus the 2-wide projected head table T."""
    nidx = nblk * 128
    S16 = nidx // 16
    nc = bacc.Bacc("TRN2", target_bir_lowering=False, debug=False,
                   num_devices=CORES)
    tbl = nc.dram_tensor("tbl", [N + 1, HID], BF16, kind="ExternalInput")
    idx = nc.dram_tensor("idx", [128, TPC * S16], I16, kind="ExternalInput")
    rr = nc.dram_tensor("rr", [128, TPC * nblk], BF16, kind="ExternalInput")
    deg = nc.dram_tensor("degc", [128, TPC], F32, kind="ExternalInput")
    # W1.T @ w_alpha / w_beta, [2 heads x 256], host-prepped tiny weights
    wp = nc.dram_tensor("wp", [1, 2, HID], F32, kind="ExternalInput")
    tc_out = nc.dram_tensor("tc_out", [128, 2, TPC], F32,
                            kind="ExternalOutput")

    with tile.TileContext(nc) as tc:
        with (
            tc.tile_pool(name="pool", bufs=1) as pool,
            tc.tile_pool(name="gat", bufs=3) as gat,
            tc.tile_pool(name="work", bufs=2) as work,
            tc.tile_pool(name="psum", bufs=2, space=MS.PSUM) as pps,
        ):
            for _rep in range(reps):
                idx_sb = pool.tile([128, TPC * S16], I16, tag="idx")
                nc.sync.dma_start(idx_sb[:], idx[:])
                rr_sb = pool.tile([128, TPC * nblk], BF16, tag="rr")
                nc.sync.dma_start(rr_sb[:], rr[:])
                norm_sb = _norm_tiles(nc, pool, deg)

                iotw = pool.tile([128, nblk, 128], BF16, tag="iotw")
                nc.gpsimd.iota(iotw[:], [[0, nblk], [1, 128]],
                               channel_multiplier=0,
                               allow_small_or_imprecise_dtypes=True)

                # broadcast the projected head weights to all partitions
                wp_sb = pool.tile([1, 2, HID], F32, tag="wp")
                nc.sync.dma_start(wp_sb[:], wp[:])
                ones_sb = pool.tile([1, 128], F32, tag="ones")
                nc.vector.memset(ones_sb[:], 1.0)
                head_b = pool.tile([128, 2, HID], F32, tag="headb")
                for hd in range(2):
                    pb = pps.tile([128, HID], F32, tag="pb")
                    nc.tensor.matmul(pb[:], ones_sb[:], wp_sb[:, hd, :],
                                     start=True, stop=True)
                    nc.vector.tensor_copy(head_b[:, hd, :], pb[:])
                t_all = pool.tile([128, 2, TPC], F32, tag="tall")

                for t in range(TPC):
                    g = gat.tile([128, nblk, HID], BF16, tag="g")
                    # <=1024 idxs per gather: SWDGE ring is 1024 descriptors
                    for c0 in range(0, nblk, 8):
                        cb = min(8, nblk - c0)
                        nc.gpsimd.dma_gather(
                            g[:, c0:c0 + cb, :], tbl[:, :],
                            idx_sb[:, t * S16 + c0 * 8:
                                   t * S16 + (c0 + cb) * 8],
                            num_idxs=cb * 128, num_idxs_reg=cb * 128,
                            elem_size=HID,
                        )
                    oall = gat.tile([128, nblk, 128], BF16, tag="o")
                    rr_bc = rr_sb[:, bass.ts(t, nblk)].unsqueeze(2)\
                        .broadcast_to([128, nblk, 128])
                    nc.vector.tensor_tensor(oall[:], rr_bc, iotw[:],
                                            op=AOT.is_equal)
                    ps = pps.tile([128, HID], F32, tag="ps")
                    for b in range(nblk):
                        nc.tensor.matmul(ps[:], oall[:, b, :], g[:, b, :],
                                         start=(b == 0), stop=(b == nblk - 1))

                    nt = norm_sb[:, t:t + 1]
                    # h0s = norm * relu(norm * S1)
                    rt = work.tile([128, HID], F32, tag="rt")
                    nc.scalar.activation(rt[:], ps[:], AFT.Relu, scale=nt)
                    h0t = work.tile([128, HID], F32, tag="h0t")
                    nc.vector.tensor_scalar(h0t[:], rt[:], nt, None,
                                            op0=AOT.mult)
                    # T[d, hd] = h0s[d, :] . wp[hd, :]
                    junk = work.tile([128, HID], F32, tag="junk")
                    for hd in range(2):
                        nc.vector.tensor_tensor(junk[:], h0t[:],
                                                head_b[:, hd, :],
                                                op=AOT.mult)
                        nc.vector.tensor_reduce(t_all[:, hd, t:t + 1],
                                                junk[:],
                                                mybir.AxisListType.X,
                                                AOT.add)
                nc.sync.dma_start(tc_out[:], t_all[:])
    nc.compile()
    return nc


def _build_l3(nblk, reps=1):
    """SPMM layer 2 on the slim 2-wide T table (padded to 128 bf16 per
    node for the 256B dma_gather floor) + ShiftedELU heads."""
    nidx = nblk * 128
    S16 = nidx // 16
    nc = bacc.Bacc("TRN2", target_bir_lowering=False, debug=False,
                   num_devices=CORES)
    tbl = nc.dram_tensor("tbl", [N + 1, TSLIM], BF16, kind="ExternalInput")
    idx = nc.dram_tensor("idx", [128, TPC * S16], I16, kind="ExternalInput")
    rr = nc.dram_tensor("rr", [128, TPC * nblk], BF16, kind="ExternalInput")
    deg = nc.dram_tensor("degc", [128, TPC], F32, kind="ExternalInput")
    bab = nc.dram_tensor("bab", [1, 2], F32, kind="ExternalInput")
    abc = nc.dram_tensor("abc", [128, 2, TPC], F32, kind="ExternalOutput")

    with tile.TileContext(nc) as tc:
        with (
            tc.tile_pool(name="pool", bufs=1) as pool,
            tc.tile_pool(name="gat", bufs=3) as gat,
            tc.tile_pool(name="work", bufs=2) as work,
            tc.tile_pool(name="psum", bufs=4, space=MS.PSUM) as pps,
        ):
            for _rep in range(reps):
                idx_sb = pool.tile([128, TPC * S16], I16, tag="idx")
                nc.sync.dma_start(idx_sb[:], idx[:])
                rr_sb = pool.tile([128, TPC * nblk], BF16, tag="rr")
                nc.sync.dma_start(rr_sb[:], rr[:])
                norm_sb = _norm_tiles(nc, pool, deg)

                iotw = pool.tile([128, nblk, 128], BF16, tag="iotw")
                nc.gpsimd.iota(iotw[:], [[0, nblk], [1, 128]],
                               channel_multiplier=0,
                               allow_small_or_imprecise_dtypes=True)

                ones_sb = pool.tile([1, 128], F32, tag="ones")
                nc.vector.memset(ones_sb[:], 1.0)
                bab_sb = pool.tile([1, 2], F32, tag="bab")
                nc.sync.dma_start(bab_sb[:], bab[:])
                bab_b = pool.tile([128, 2], F32, tag="babb")
                pbb = pps.tile([128, 2], F32, tag="pbb")
                nc.tensor.matmul(pbb[:], ones_sb[:], bab_sb[:],
                                 start=True, stop=True)
                nc.vector.tensor_copy(bab_b[:], pbb[:])
                ab_all = pool.tile([128, 2, TPC], F32, tag="aball")

                for t in range(TPC):
                    g = gat.tile([128, nblk, TSLIM], BF16, tag="g")
                    # <=1024 idxs per gather: SWDGE ring is 1024 descriptors
                    for c0 in range(0, nblk, 8):
                        cb = min(8, nblk - c0)
                        nc.gpsimd.dma_gather(
                            g[:, c0:c0 + cb, :], tbl[:, :],
                            idx_sb[:, t * S16 + c0 * 8:
                                   t * S16 + (c0 + cb) * 8],
                            num_idxs=cb * 128, num_idxs_reg=cb * 128,
                            elem_size=TSLIM,
                        )
                    oall = gat.tile([128, nblk, 128], BF16, tag="o")
                    rr_bc = rr_sb[:, bass.ts(t, nblk)].unsqueeze(2)\
                        .broadcast_to([128, nblk, 128])
                    nc.vector.tensor_tensor(oall[:], rr_bc, iotw[:],
                                            op=AOT.is_equal)
                    ps = pps.tile([128, 2], F32, tag="ps")
                    for b in range(nblk):
                        nc.tensor.matmul(ps[:], oall[:, b, :],
                                         g[:, b, 0:2],
                                         start=(b == 0), stop=(b == nblk - 1))

                    nt = norm_sb[:, t:t + 1]
                    x = work.tile([128, 2], F32, tag="x")
                    nc.vector.tensor_scalar(x[:], ps[:], nt, None,
                                            op0=AOT.mult)
                    nc.vector.tensor_tensor(x[:], x[:], bab_b[:],
                                            op=AOT.add)
                    # elu(x) + 1.5 = exp(min(x,0)) + max(x,0) + 0.5
                    mn = work.tile([128, 2], F32, tag="mn")
                    nc.vector.tensor_scalar(mn[:], x[:], 0.0, None,
                                            op0=AOT.min)
                    ex = work.tile([128, 2], F32, tag="ex")
                    nc.scalar.activation(ex[:], mn[:], AFT.Exp)
                    mx = work.tile([128, 2], F32, tag="mx")
                    nc.vector.tensor_scalar(mx[:], x[:], 0.0, 0.5,
                                            op0=AOT.max, op1=AOT.add)
                    nc.vector.tensor_tensor(ab_all[:, :, t], ex[:],
                                            mx[:], op=AOT.add)
                nc.sync.dma_start(abc[:], ab_all[:])
    nc.compile()
    return nc


def _build_l4(reps=1):
    """Pairwise broadcast-sum panels in bf16, OBATCH tiles per DMA."""
    NG = TPC // OBATCH
    nc = bacc.Bacc("TRN2", target_bir_lowering=False, debug=False,
                   num_devices=CORES)
    af = nc.dram_tensor("af", [1, N], BF16, kind="ExternalInput")
    bf = nc.dram_tensor("bf", [1, N], BF16, kind="ExternalInput")
    act = nc.dram_tensor("act", [128, TPC], F32, kind="ExternalInput")
    bct = nc.dram_tensor("bct", [128, TPC], F32, kind="ExternalInput")
    # [group, partition, slot, col]; host reassembles row order
    arows = nc.dram_tensor("arows", [NG, 128, OBATCH, N], BF16,
                           kind="ExternalOutput")
    brows = nc.dram_tensor("brows", [NG, 128, OBATCH, N], BF16,
                           kind="ExternalOutput")

    with tile.TileContext(nc) as tc:
        with (
            tc.tile_pool(name="pool", bufs=1) as pool,
            tc.tile_pool(name="out", bufs=2) as outp,
            tc.tile_pool(name="psum", bufs=4, space=MS.PSUM) as pps,
        ):
            for _rep in range(reps):
                ones_sb = pool.tile([1, 128], BF16, tag="ones")
                nc.vector.memset(ones_sb[:], 1.0)
                act_sb = pool.tile([128, TPC], F32, tag="act")
                nc.sync.dma_start(act_sb[:], act[:])
                bct_sb = pool.tile([128, TPC], F32, tag="bct")
                nc.sync.dma_start(bct_sb[:], bct[:])

                full_b = {}
                for name, src in (("a", af), ("b", bf)):
                    row = pool.tile([1, N], BF16, tag="row")
                    nc.sync.dma_start(row[:], src[:])
                    bcast = pool.tile([128, N], BF16, tag=f"bc{name}")
                    for s in range(N // 512):
                        pbc = pps.tile([128, 512], F32, tag="pbc")
                        nc.tensor.matmul(pbc[:], ones_sb[:],
                                         row[:, bass.ts(s, 512)],
                                         start=True, stop=True)
                        nc.scalar.activation(bcast[:, bass.ts(s, 512)],
                                             pbc[:], AFT.Copy)
                    full_b[name] = bcast

                for gidx in range(NG):
                    for name, scal, dst in (("a", act_sb, arows),
                                            ("b", bct_sb, brows)):
                        o = outp.tile([128, OBATCH, N], BF16, tag="o")
                        for s in range(OBATCH):
                            t = gidx * OBATCH + s
                            nc.vector.tensor_scalar(o[:, s, :],
                                                    full_b[name][:],
                                                    scal[:, t:t + 1], None,
                                                    op0=AOT.add)
                        nc.sync.dma_start(dst[gidx], o[:])
    nc.compile()
    return nc


def _prep_edges(row, col):
    """Shard edges by dest row-tile; pad each (core, tile) group to a
    common multiple of 128 edges (pad gathers the zero row N)."""
    g = row >> 7                       # global 128-row tile id, 0..63
    # sort by (dest tile, src col): col-sorted groups make the gather's
    # HBM reads near-sequential instead of random
    order = np.lexsort((col, g))
    gs = g[order]
    col_s = col[order].astype(np.int32)
    rr_s = (row[order] & 127).astype(np.int32)
    counts = np.bincount(gs, minlength=N // 128)
    nblk = int(np.ceil(counts.max() / 128))
    nidx = nblk * 128
    starts = np.concatenate([[0], np.cumsum(counts)])

    idx_pad = np.full((N // 128, nidx), N, np.int32)
    rr_pad = np.zeros((N // 128, nidx), np.int32)
    for gt in range(N // 128):
        s, e = starts[gt], starts[gt + 1]
        idx_pad[gt, : e - s] = col_s[s:e]
        rr_pad[gt, : e - s] = rr_s[s:e]

    idx_planes = []
    rr_mats = []
    for c in range(CORES):
        planes = []
        rrcols = []
        for t in range(TPC):
            gt = c * TPC + t
            planes.append(idx_pad[gt].reshape(-1, 16).T)      # (16, nidx/16)
            rrcols.append(rr_pad[gt].reshape(nblk, 128).T)    # (128, nblk)
        plane = np.tile(np.hstack(planes), (8, 1)).astype(np.int16)
        idx_planes.append(np.ascontiguousarray(plane))
        rr_mats.append(np.ascontiguousarray(np.hstack(rrcols)))
    return nblk, idx_planes, rr_mats


_cache = {}


def _get(name, builder, *args):
    key = (name,) + args
    if key not in _cache:
        _cache[key] = builder(*args)
    return _cache[key]


def kernel(row, col, h, W0, W1, w_alpha, b_alpha, w_beta, b_beta):
    LAST_LAUNCHES.clear()
    row = np.asarray(row)
    col = np.asarray(col)
    h = np.asarray(h, np.float32)
    W0 = np.asarray(W0, np.float32)
    W1 = np.asarray(W1, np.float32)

    deg = np.bincount(row, minlength=N).astype(np.float32)
    degc = [np.ascontiguousarray(deg[c * RPC:(c + 1) * RPC]
                                 .reshape(TPC, 128).T) for c in range(CORES)]
    nblk, idx_planes, rr_mats = _prep_edges(row, col)

    # ---- L1: X1 = norm * (h @ W0.T) ----
    hT = np.ascontiguousarray(h.T.astype(NP_BF16))     # (512, 8192)
    w0t = np.ascontiguousarray(
        W0.T.astype(NP_BF16).reshape(F_IN // 128, 128, HID))
    nc1 = _get("l1", _build_l1)
    in1 = [{
        "ht": np.ascontiguousarray(
            hT[:, c * RPC:(c + 1) * RPC].reshape(F_IN // 128, 128, RPC)),
        "w0t": w0t,
        "degc": degc[c],
    } for c in range(CORES)]
    r1 = _run(nc1, in1, "l1", _build_l1)
    table1 = np.vstack([np.concatenate([r1[c]["x1c"] for c in range(CORES)]),
                        np.zeros((1, HID), NP_BF16)])

    # ---- L2: h0s + projected head table T ----
    nc2 = _get("l2", _build_l2, nblk)
    wp_in = np.ascontiguousarray(
        (W1.T @ np.stack([np.asarray(w_alpha, np.float32),
                          np.asarray(w_beta, np.float32)], axis=1))
        .T.reshape(1, 2, HID).astype(np.float32))
    rr_bf = [np.ascontiguousarray(m.astype(NP_BF16)) for m in rr_mats]
    in2 = [{"tbl": table1, "idx": idx_planes[c], "rr": rr_bf[c],
            "degc": degc[c], "wp": wp_in} for c in range(CORES)]
    r2 = _run(nc2, in2, "l2", lambda reps=1: _build_l2(nblk, reps))
    # tc_out[p, hd, t] -> T value for node c*RPC + t*128 + p
    T = np.zeros((N + 1, TSLIM), NP_BF16)
    for c in range(CORES):
        tc_c = r2[c]["tc_out"]                      # [128, 2, TPC]
        T[c * RPC:(c + 1) * RPC, 0:2] = \
            tc_c.transpose(2, 0, 1).reshape(RPC, 2)

    # ---- L3: alpha/beta = elu(norm * (A @ T) + b) + 1.5 ----
    nc3 = _get("l3", _build_l3, nblk)
    bab_in = np.array([[np.float32(np.asarray(b_alpha).reshape(-1)[0]),
                        np.float32(np.asarray(b_beta).reshape(-1)[0])]],
                      np.float32)
    in3 = [{"tbl": T, "idx": idx_planes[c], "rr": rr_bf[c],
            "degc": degc[c], "bab": bab_in} for c in range(CORES)]
    r3 = _run(nc3, in3, "l3", lambda reps=1: _build_l3(nblk, reps))
    # abc[p, hd, t] -> value for node c*RPC + t*128 + p
    alpha = np.concatenate(
        [r3[c]["abc"][:, 0, :].T.reshape(-1) for c in range(CORES)])
    beta = np.concatenate(
        [r3[c]["abc"][:, 1, :].T.reshape(-1) for c in range(CORES)])

    # ---- L4: pairwise broadcast-sum panels (bf16 on device) ----
    nc4 = _get("l4", _build_l4)
    af = np.ascontiguousarray(alpha.reshape(1, N).astype(NP_BF16))
    bf = np.ascontiguousarray(beta.reshape(1, N).astype(NP_BF16))
    in4 = [{
        "af": af, "bf": bf,
        "act": np.ascontiguousarray(
            alpha[c * RPC:(c + 1) * RPC].reshape(TPC, 128).T),
        "bct": np.ascontiguousarray(
            beta[c * RPC:(c + 1) * RPC].reshape(TPC, 128).T),
    } for c in range(CORES)]
    r4 = _run(nc4, in4, "l4", _build_l4)

    def _panels(key):
        # [NG, 128, OBATCH, N] -> row-major [RPC, N] f32
        parts = []
        for c in range(CORES):
            a = r4[c][key].transpose(0, 2, 1, 3).reshape(RPC, N)
            parts.append(a.astype(np.float32))
        return np.concatenate(parts)

    return _panels("arows"), _panels("brows")
